# revision 41
# baseline (speedup 1.0000x reference)
"""T5-style encoder self-attention (dense_transformer) on 8 Trainium2 NeuronCores.

Problem (full shapes): hidden [2,2048,2048], Wq/Wk/Wv/Wo [2048,2048],
rel_emb [32,32] (bidirectional T5 relative-position bias), mask [2,1,1,2048].

Sharding: data-parallel over batch (2) x tensor-parallel over heads (4 groups
of 8 heads) = 8 cores, Megatron-style. Each core computes a partial output
[2048,2048] for its batch (its 8 heads through its Wo row-slice); the host
sums 4 partials per batch.

Per-core kernel design (bf16 operands, fp32 PSUM accumulation):
  - The exp'd relative-position bias diagonals are computed on the HOST
    (structural bucket table x rel_emb gather + exp -> [8, 4096] bf16) and
    DMA'd in; per-head Toeplitz tiles U are built with one sheared DMA each.
  - Q^T is stored with s REVERSED so the bias becomes a positive-shear
    Toeplitz.
  - Phase A: ONE streaming pass over x^T per s-chunk computes Q^T/K^T (head
    pair 0) AND V (all heads) -> PE ~100% busy, x^T read once here.
  - Phase B: per (pair, qc) attention kt-loop; scores via row-packed pair of
    K=64 matmuls (both PE-array halves concurrently); ONE ACT exp per kt
    covers both heads [128,1024] (ACT cadence ~1.2us/iter); DVE multiplies
    by the Toeplitz exp-bias; PV uses V_aug=[V | ones] so psum rows 64:128
    carry the softmax denominator for free.  The NEXT pair's Q/K projection
    matmuls are interleaved one kd-step per kt-iteration (keeps the PE
    queue dense at full p-state); their psum evictions run at the producing
    chunk's tail (deferring them across the chunk boundary races with the
    ring-1 psum slot reuse and must NOT be done).
  - Softmax normalization is deferred: denominator rows -> DRAM, DVE
    reciprocal, broadcast back via DRAM, multiplied into ctxt -- pipelined
    two chunks behind so no engine waits on the DMA round trips.
  - Phase C: output projection immediately after the last attention qc;
    psum evictions alternate ACT/DVE; chunk order puts the last-normalized
    q-range last.
"""

import math
import sys

for _p in ("/opt/trn_rl_repo",):
    if _p not in sys.path:
        sys.path.insert(0, _p)

import numpy as np

import concourse.bass as bass
import concourse.mybir as mybir
import concourse.tile as tile
from concourse import bacc
from concourse.bass_utils import run_bass_kernel_spmd

DT = mybir.dt
AF = mybir.ActivationFunctionType
OP = mybir.AluOpType

# ---- problem constants (hardcoded per contract) ----
B, S, D = 2, 2048, 2048
N_HEADS, D_KV = 32, 64
NUM_BUCKETS, MAX_DISTANCE = 32, 128
NCORES = 8
HL = 8            # heads per core
P = 128
SC = 512          # free-dim chunk
NKT = S // P      # 16 k-tiles
NQC = S // SC     # 4 q-chunks
NDT = D // P      # 16 D-tiles
NMT = (HL * D_KV) // P   # 4 hd m-tiles per core
W_U = 3968        # toeplitz tile width: max j0 (=15*128+3*512=3456) + 512
NDIAG = 4096      # ud row stride


def _rel_bucket_host(d):
    """Exact numpy replica of reference._relative_position_bucket (fp32 math,
    int32 truncation) for bidirectional buckets. d = k - q (int array)."""
    num_buckets = NUM_BUCKETS // 2          # 16
    max_exact = num_buckets // 2            # 8
    rel = np.asarray(d, dtype=np.int64)
    buckets = (rel > 0).astype(np.int32) * num_buckets
    arel = np.abs(rel)
    is_small = arel < max_exact
    rp_safe = np.maximum(arel, 1).astype(np.float32)
    log_ratio = np.log(rp_safe / np.float32(max_exact)).astype(np.float32)
    scale = np.float32(math.log(MAX_DISTANCE / max_exact))
    rp_large = max_exact + (log_ratio / scale * np.float32(num_buckets - max_exact)).astype(np.int32)
    rp_large = np.minimum(rp_large, num_buckets - 1)
    buckets = buckets + np.where(is_small, arel.astype(np.int32), rp_large)
    return buckets.astype(np.int32)


_BUCKETS = _rel_bucket_host(np.arange(NDIAG - 1) - (S - 1))  # structural


def _build(debug=False):
    nc = bacc.Bacc(None, name="attn_tp2")

    xt = nc.declare_dram_parameter("xt", [D, S], DT.bfloat16, isOutput=False)
    wq = nc.declare_dram_parameter("wq", [D, HL * D_KV], DT.bfloat16, isOutput=False)
    wk = nc.declare_dram_parameter("wk", [D, HL * D_KV], DT.bfloat16, isOutput=False)
    wv = nc.declare_dram_parameter("wv", [D, HL * D_KV], DT.bfloat16, isOutput=False)
    wo = nc.declare_dram_parameter("wo", [HL * D_KV, D], DT.bfloat16, isOutput=False)
    mask = nc.declare_dram_parameter("mask", [S], DT.float32, isOutput=False)
    ud = nc.declare_dram_parameter("ud", [HL, NDIAG], DT.bfloat16, isOutput=False)
    out = nc.declare_dram_parameter("out", [S, D], DT.bfloat16, isOutput=True)
    if debug:
        dbg_qt = nc.declare_dram_parameter("dbg_qt", [P, NMT * S], DT.bfloat16, isOutput=True)
        dbg_kt = nc.declare_dram_parameter("dbg_kt", [P, NMT * S], DT.bfloat16, isOutput=True)
        dbg_va = nc.declare_dram_parameter("dbg_va", [P, NKT * HL * 2 * D_KV], DT.bfloat16, isOutput=True)
        dbg_cx = nc.declare_dram_parameter("dbg_cx", [P, NMT * S], DT.bfloat16, isOutput=True)
        dbg_rc = nc.declare_dram_parameter("dbg_rc", [HL * NQC, SC], DT.float32, isOutput=True)

    with tile.TileContext(nc) as tc:
        with (
            tc.tile_pool(name="res", bufs=1) as res,          # persistent tensors
            tc.tile_pool(name="xtp", bufs=8) as xtp,         # x^T stream tiles
            tc.tile_pool(name="stage", bufs=2) as stage,      # staging
            tc.tile_pool(name="upool", bufs=3) as upool,      # toeplitz exp-bias tiles
            tc.tile_pool(name="pexp", bufs=3) as pexpp,       # probs tiles
            tc.tile_pool(name="outp", bufs=7) as outp,        # out staging
            tc.tile_pool(name="psum", bufs=1, space="PSUM") as psum,
            tc.tile_pool(name="dram", bufs=1, space="DRAM") as dramp,
        ):
            # psum tags: "s" ring2 [128,1024] + "pj" ring1 [128,1024] +
            # "cx0"/"cx1" ring1 [128,512] -> exactly 8 banks.  Separate cx
            # tags release each head's PV accumulator after its own 2 tail
            # reads (~1.3us) instead of all 4 (~2.6us), shrinking the
            # chunk-boundary stall on the next chunk's first PV matmul.
            def ps_tile(tag, name):
                if tag in ("cx0", "cx1"):
                    return psum.tile([P, SC], DT.float32, tag=tag, name=name,
                                     bufs=1)
                return psum.tile([P, 2 * SC], DT.float32, tag=tag, name=name,
                                 bufs=2 if tag == "s" else 1)

            # ---------- constants ----------
            # vaug ones-block init (overlaps the initial weight DMAs)
            vaug = res.tile([P, NKT, HL, 2 * D_KV], DT.bfloat16, tag="vaug")
            nc.vector.memset(vaug[:], 1.0)

            mask_sb = res.tile([P, NKT], DT.float32, tag="mask")
            nc.sync.dma_start(mask_sb[:], mask.ap().rearrange("(kt p) -> p kt", p=P))

            den_dram = dramp.tile([HL * NQC, SC], DT.float32)
            rcp_dram = dramp.tile([HL * NQC, SC], DT.float32)

            # weights (resident, bf16); interleave per-kd chunks with the
            # first s-chunk's x^T tiles so phase A's PE can start within ~2us.
            wq_sb = res.tile([P, NDT, HL * D_KV], DT.bfloat16, tag="wq")
            wk_sb = res.tile([P, NDT, HL * D_KV], DT.bfloat16, tag="wk")
            wv_sb = res.tile([P, NDT, HL * D_KV], DT.bfloat16, tag="wv")
            wo_sb = res.tile([P, NMT, D], DT.bfloat16, tag="wo")
            pre_xt = {}
            for kd in range(NDT):
                if kd < 8:
                    t = xtp.tile([P, SC], DT.bfloat16, tag="xt",
                                 name=f"xa0_{kd}")
                    nc.sync.dma_start(t[:], xt[kd * P:(kd + 1) * P, 0:SC])
                    pre_xt[kd] = t
                # phase A needs only pair 0's columns of Wq/Wk; the rest is
                # for the interleaved projections ~80us later (loaded below)
                nc.sync.dma_start(wq_sb[:, kd, 0:P], wq[kd * P:(kd + 1) * P, 0:P])
                nc.sync.dma_start(wk_sb[:, kd, 0:P], wk[kd * P:(kd + 1) * P, 0:P])
                nc.sync.dma_start(wv_sb[:, kd, :], wv[kd * P:(kd + 1) * P, :])

            # persistent activations
            qt_sb = res.tile([P, NMT, S], DT.bfloat16, tag="qt")   # q REVERSED
            kt_sb = res.tile([P, NMT, S], DT.bfloat16, tag="kt")
            ctxt = res.tile([P, NMT, S], DT.bfloat16, tag="ctxt")

            def load_u(pr):
                u_t = {}
                for hh in (2 * pr, 2 * pr + 1):
                    u = upool.tile([P, W_U], DT.bfloat16, tag="u", name=f"u{hh}")
                    uda = ud.ap()
                    shear = bass.AP(
                        tensor=uda.tensor,
                        offset=uda.offset + hh * NDIAG,
                        ap=[[1, P], [1, W_U]],
                    )
                    nc.sync.dma_start(u[:], shear)
                    u_t[hh] = u
                return u_t

            def rev_ap(base, start_col, total):
                """AP over `base` writing SC columns reversed: column j of the
                source lands at logical position total-1-(start_col+j)."""
                return bass.AP(
                    tensor=base.tensor,
                    offset=base.offset + (total - 1 - start_col),
                    ap=[list(base.ap[0]), [-1, SC]],
                )

            # ---------- phase A: fused pass: Q/K (pair 0) + V (all heads) ----
            for nq in range(NQC):
                qk_ps = ps_tile("s", f"aqk{nq}")
                q_ps, k_ps = qk_ps[:, 0:SC], qk_ps[:, SC:2 * SC]
                v0p = ps_tile("cx0", f"av0_{nq}")
                v1p = ps_tile("cx1", f"av1_{nq}")
                v23 = ps_tile("pj", f"av23_{nq}")
                v_ps = [v0p, v1p, v23[:, 0:SC], v23[:, SC:2 * SC]]
                for kd in range(NDT):
                    if nq == 0 and kd in pre_xt:
                        xt_t = pre_xt.pop(kd)
                    else:
                        xt_t = xtp.tile([P, SC], DT.bfloat16, tag="xt",
                                        name=f"xa{nq}_{kd}")
                        nc.sync.dma_start(
                            xt_t[:],
                            xt[kd * P:(kd + 1) * P, nq * SC:(nq + 1) * SC]
                        )
                    nc.tensor.matmul(
                        q_ps, wq_sb[:, kd, 0:P], xt_t[:],
                        start=(kd == 0), stop=(kd == NDT - 1),
                    )
                    nc.tensor.matmul(
                        k_ps, wk_sb[:, kd, 0:P], xt_t[:],
                        start=(kd == 0), stop=(kd == NDT - 1),
                    )
                    for st in range(4):
                        nc.tensor.matmul(
                            v_ps[st], xt_t[:, st * P:(st + 1) * P],
                            wv_sb[:, kd, :],
                            start=(kd == 0), stop=(kd == NDT - 1),
                        )
                # evictions: v0/v1 first (their psum slots are needed soonest)
                for st in range(4):
                    kt_glob = nq * 4 + st
                    nc.vector.tensor_copy(
                        vaug[:, kt_glob, :, 0:D_KV],
                        v_ps[st].rearrange("p (h d) -> p h d", d=D_KV),
                    )
                nc.vector.tensor_copy(rev_ap(qt_sb[:, 0, :], nq * SC, S), q_ps)
                nc.vector.tensor_copy(kt_sb[:, 0, nq * SC:(nq + 1) * SC], k_ps)

            # phase-B-only inputs: emitted after ALL of phase A's DMA
            # traffic (anything placed earlier delays nq>=1 xt streams by
            # queueing in front of them).  u tiles first (needed at the very
            # start of phase B), then the projection weight remainders, wo.
            u_t = load_u(0)
            for kd in range(NDT):
                nc.sync.dma_start(wq_sb[:, kd, P:HL * D_KV],
                                  wq[kd * P:(kd + 1) * P, P:HL * D_KV])
                nc.sync.dma_start(wk_sb[:, kd, P:HL * D_KV],
                                  wk[kd * P:(kd + 1) * P, P:HL * D_KV])
            for mt in range(NMT):
                nc.sync.dma_start(wo_sb[:, mt, :],
                                  wo[mt * P:(mt + 1) * P, :])

            # ---------- phase B ----------
            def attn_qc(pr, qc, u_t, proj_pr, pnq, dve_slots):
                """Attention for head pair pr, reversed-col chunk qc.
                proj_pr/pnq: pair + s-chunk whose Q/K projection kd-steps
                interleave here (starting at kt=3).  dve_slots: kt -> list of
                thunks emitting deferred non-critical DVE work right after
                that iteration's critical multiplies."""
                h0, h1 = 2 * pr, 2 * pr + 1
                jg0 = qc * SC
                cx0 = ps_tile("cx0", f"cx0_{pr}_{qc}")
                cx1 = ps_tile("cx1", f"cx1_{pr}_{qc}")
                if proj_pr is not None:
                    jp0 = pnq * SC
                    pj_ps = ps_tile("pj", f"pj{proj_pr}_{pnq}")
                    pq_ps, pk_ps = pj_ps[:, 0:SC], pj_ps[:, SC:2 * SC]
                    pxt = {}
                    for kd in range(2):
                        t = xtp.tile([P, SC], DT.bfloat16, tag="xt",
                                     name=f"xp{proj_pr}_{pnq}_{kd}")
                        nc.sync.dma_start(
                            t[:], xt[kd * P:(kd + 1) * P, jp0:jp0 + SC])
                        pxt[kd] = t
                    # kd steps per kt iteration (start at 2, catch up at end):
                    # kt 2..13 -> kd 0..11 ; 14 -> 12,13 ; 15 -> 14,15
                    kd_sched = {kt: [kt - 2] for kt in range(2, NKT - 2)}
                    kd_sched[NKT - 2] = [NKT - 4, NKT - 3]
                    kd_sched[NKT - 1] = [NKT - 2, NKT - 1]

                def emit_qk(kt):
                    s01 = ps_tile("s", f"s{pr}_{qc}_{kt}")
                    nc.tensor.matmul(
                        s01[:, 0:SC], kt_sb[0:64, pr, kt * P:(kt + 1) * P],
                        qt_sb[0:64, pr, jg0:jg0 + SC],
                        start=True, stop=True, tile_position=(0, 0),
                    )
                    nc.tensor.matmul(
                        s01[:, SC:2 * SC], kt_sb[64:128, pr, kt * P:(kt + 1) * P],
                        qt_sb[64:128, pr, jg0:jg0 + SC],
                        start=True, stop=True, tile_position=(64, 0),
                    )
                    return s01

                s01 = emit_qk(0)
                for kt in range(NKT):
                    s01_next = emit_qk(kt + 1) if kt + 1 < NKT else None
                    px = pexpp.tile([P, 2 * SC], DT.bfloat16, tag="pexp",
                                    name=f"px{pr}_{qc}_{kt}")
                    nc.scalar.activation(
                        out=px[:], in_=s01[:], func=AF.Exp,
                        bias=mask_sb[:, kt:kt + 1], scale=1.0 / math.sqrt(D_KV),
                    )
                    j0 = kt * P + jg0
                    nc.vector.tensor_tensor(
                        px[:, 0:SC], px[:, 0:SC], u_t[h0][:, j0:j0 + SC], OP.mult
                    )
                    nc.vector.tensor_tensor(
                        px[:, SC:2 * SC], px[:, SC:2 * SC],
                        u_t[h1][:, j0:j0 + SC], OP.mult
                    )
                    for thunk in dve_slots.get(kt, ()):
                        thunk()
                    if proj_pr is not None:
                        for kd in kd_sched.get(kt, ()):
                            nc.tensor.matmul(
                                pq_ps, wq_sb[:, kd, proj_pr * P:(proj_pr + 1) * P],
                                pxt[kd][:],
                                start=(kd == 0), stop=(kd == NDT - 1),
                            )
                            nc.tensor.matmul(
                                pk_ps, wk_sb[:, kd, proj_pr * P:(proj_pr + 1) * P],
                                pxt[kd][:],
                                start=(kd == 0), stop=(kd == NDT - 1),
                            )
                            del pxt[kd]
                            nkd = kd + 2
                            if nkd < NDT:
                                t = xtp.tile([P, SC], DT.bfloat16, tag="xt",
                                             name=f"xp{proj_pr}_{pnq}_{nkd}")
                                nc.sync.dma_start(
                                    t[:],
                                    xt[nkd * P:(nkd + 1) * P, jp0:jp0 + SC])
                                pxt[nkd] = t
                    nc.tensor.matmul(
                        cx0, vaug[:, kt, h0, :], px[:, 0:SC],
                        start=(kt == 0), stop=(kt == NKT - 1),
                    )
                    nc.tensor.matmul(
                        cx1, vaug[:, kt, h1, :], px[:, SC:2 * SC],
                        start=(kt == 0), stop=(kt == NKT - 1),
                    )
                    s01 = s01_next

                # ctx eviction (unnormalized, un-reversing q) + denominator
                # rows un-reversed into SBUF; reciprocal + DRAM broadcast is
                # deferred to the NEXT chunk's DVE slots.
                dns = []
                for hh, cx in ((h0, cx0), (h1, cx1)):
                    off = (hh % 2) * 64
                    base = ctxt[off:off + 64, pr, :]
                    nc.vector.tensor_copy(rev_ap(base, jg0, S), cx[0:D_KV, :])
                    dn = stage.tile([P, SC], DT.float32, tag="dn",
                                    name=f"dn{hh}_{qc}", bufs=2)
                    nc.vector.tensor_copy(
                        rev_ap(dn[64:65, :], 0, SC), cx[64:65, :])
                    nc.sync.dma_start(den_dram[hh * NQC + qc, :], dn[64:65, :])
                    dns.append(dn)

                # projection psum evictions after the cx evictions (cx slot
                # is needed by the next chunk's first PV; the pj slot only by
                # its kt=2 projection matmul).  Same-chunk emission: deferring
                # these across the boundary races with ring-1 slot reuse.
                if proj_pr is not None:
                    nc.vector.tensor_copy(
                        rev_ap(qt_sb[:, proj_pr, :], jp0, S), pq_ps)
                    nc.vector.tensor_copy(
                        kt_sb[:, proj_pr, jp0:jp0 + SC], pk_ps)

                return None, None

            def norm_fetch(pr, qc):
                rows = [2 * pr * NQC + qc, (2 * pr + 1) * NQC + qc]
                den2 = stage.tile([2, SC], DT.float32, tag="den2",
                                  name=f"de{pr}_{qc}", bufs=1)
                for r, row in enumerate(rows):
                    nc.sync.dma_start(den2[r:r + 1, :], den_dram[row, :])
                return {"pr": pr, "qc": qc, "rows": rows, "den2": den2}

            def norm_rcp(rec):
                rcp2 = stage.tile([2, SC], DT.float32, tag="rcp2",
                                  name=f"rc{rec['pr']}_{rec['qc']}", bufs=1)
                nc.vector.reciprocal_approx_fast(
                    out=rcp2[:], in_=rec["den2"][:])
                rbs = []
                for r, row in enumerate(rec["rows"]):
                    nc.sync.dma_start(rcp_dram[row, :], rcp2[r:r + 1, :])
                    off = r * 64
                    rb = stage.tile([P, SC], DT.float32, tag="rb",
                                    name=f"rb{rec['pr']}_{rec['qc']}_{r}", bufs=4)
                    bcast = bass.AP(
                        tensor=rcp_dram.tensor,
                        offset=rcp_dram.offset + row * SC,
                        ap=[[0, D_KV], [1, SC]],
                    )
                    nc.sync.dma_start(rb[off:off + D_KV, :], bcast)
                    rbs.append(rb)
                rec["rbs"] = rbs

            def norm_apply(rec):
                """Multiply ctxt rows of (pr, qc) by broadcast reciprocals."""
                q0t = S - (rec["qc"] + 1) * SC
                for r in range(2):
                    off = r * 64
                    cslc = ctxt[off:off + 64, rec["pr"], q0t:q0t + SC]
                    nc.vector.tensor_tensor(
                        cslc, cslc, rec["rbs"][r][off:off + D_KV, :], OP.mult)

            nrecs = []
            prev_ev = None
            for pr in range(HL // 2):
                proj_pr = pr + 1 if pr + 1 < HL // 2 else None
                if proj_pr is not None:
                    next_u = load_u(proj_pr)
                for qc in range(NQC):
                    prev_ev, rec = attn_qc(pr, qc, u_t, proj_pr, qc, {})
                    nrecs.append(norm_fetch(pr, qc))
                    if len(nrecs) >= 2:
                        norm_rcp(nrecs[-2])
                    if len(nrecs) >= 3:
                        norm_apply(nrecs[-3])
                if proj_pr is not None:
                    u_t = next_u
            norm_rcp(nrecs[-1])
            norm_apply(nrecs[-2])
            norm_apply(nrecs[-1])  # rb DMA lands a few us into phase C

            # ---------- phase C: output projection ----------
            # ctxt columns were normalized in order qc=0..3 i.e. column
            # blocks [1536,2048), [1024,1536), ... -> last-normalized last.
            st_order = []
            for qc in range(NQC):
                q0t = S - (qc + 1) * SC
                st_order.extend(range(q0t // P, q0t // P + NQC))
            tags = ["s", "s", "pj"]
            ti = 0
            for st in st_order:
                for ndp in range(2):  # two [128,1024] psum tiles per st
                    o2 = ps_tile(tags[ti % 3], f"o{st}_{ndp}")
                    for half in range(2):
                        nd = 2 * ndp + half
                        o_ps = o2[:, half * SC:(half + 1) * SC]
                        for m in range(NMT):
                            nc.tensor.matmul(
                                o_ps, ctxt[:, m, st * P:(st + 1) * P],
                                wo_sb[:, m, nd * SC:(nd + 1) * SC],
                                start=(m == 0), stop=(m == NMT - 1),
                            )
                    # evict halves on alternating engines
                    for half in range(2):
                        nd = 2 * ndp + half
                        o_t = outp.tile([P, SC], DT.bfloat16, tag="out",
                                        name=f"ot{st}_{nd}")
                        if (ti + half) % 2 == 0:
                            nc.scalar.copy(o_t[:], o2[:, half * SC:(half + 1) * SC])
                        else:
                            nc.vector.tensor_copy(
                                o_t[:], o2[:, half * SC:(half + 1) * SC])
                        nc.sync.dma_start(
                            out[st * P:(st + 1) * P, nd * SC:(nd + 1) * SC],
                            o_t[:])
                    ti += 1

    nc.finalize()
    return nc


_NC_CACHE = None


def _get_nc():
    global _NC_CACHE
    if _NC_CACHE is None:
        _NC_CACHE = _build()
    return _NC_CACHE


def _in_maps(hidden_states, attention_mask, Wq, Wk, Wv, Wo, rel_emb):
    import ml_dtypes
    bf16 = ml_dtypes.bfloat16
    maps = []
    for c in range(NCORES):
        b, g = c // 4, c % 4
        hlo, hhi = g * HL, (g + 1) * HL
        udm = np.zeros((HL, NDIAG), dtype=np.float32)
        udm[:, :NDIAG - 1] = np.exp(rel_emb[_BUCKETS, hlo:hhi]).T
        maps.append({
            "xt": np.ascontiguousarray(hidden_states[b].T).astype(bf16),
            "wq": np.ascontiguousarray(Wq[:, hlo * D_KV:hhi * D_KV]).astype(bf16),
            "wk": np.ascontiguousarray(Wk[:, hlo * D_KV:hhi * D_KV]).astype(bf16),
            "wv": np.ascontiguousarray(Wv[:, hlo * D_KV:hhi * D_KV]).astype(bf16),
            "wo": np.ascontiguousarray(Wo[hlo * D_KV:hhi * D_KV, :]).astype(bf16),
            "mask": np.ascontiguousarray(attention_mask[b, 0, 0, :]).astype(np.float32),
            "ud": udm.astype(bf16),
        })
    return maps


def kernel(hidden_states, attention_mask, Wq, Wk, Wv, Wo, rel_emb, _trace=False,
           _trace_kwargs=None):
    hidden_states = np.asarray(hidden_states, dtype=np.float32)
    attention_mask = np.asarray(attention_mask, dtype=np.float32)
    Wq = np.asarray(Wq, dtype=np.float32)
    Wk = np.asarray(Wk, dtype=np.float32)
    Wv = np.asarray(Wv, dtype=np.float32)
    Wo = np.asarray(Wo, dtype=np.float32)
    rel_emb = np.asarray(rel_emb, dtype=np.float32)

    nc = _get_nc()
    maps = _in_maps(hidden_states, attention_mask, Wq, Wk, Wv, Wo, rel_emb)
    kw = dict(_trace_kwargs or {})
    res = run_bass_kernel_spmd(nc, maps, core_ids=list(range(NCORES)),
                               trace=_trace, **kw)
    kernel.last_results = res
    outp = np.empty((B, S, D), dtype=np.float32)
    for b in range(B):
        acc = np.asarray(res.results[4 * b]["out"], dtype=np.float32).copy()
        for g in range(1, 4):
            acc += np.asarray(res.results[4 * b + g]["out"], dtype=np.float32)
        outp[b] = acc
    return outp


# revision 43
# speedup vs baseline: 1.1941x; 1.1941x over previous
"""T5-style encoder self-attention (dense_transformer) on 8 Trainium2 NeuronCores.

Problem (full shapes): hidden [2,2048,2048], Wq/Wk/Wv/Wo [2048,2048],
rel_emb [32,32] (bidirectional T5 relative-position bias), mask [2,1,1,2048].

Sharding: data-parallel over batch (2) x tensor-parallel over heads (4 groups
of 8 heads) = 8 cores, Megatron-style. Each core computes a partial output
[2048,2048] for its batch (its 8 heads through its Wo row-slice); the host
sums 4 partials per batch.

Per-core kernel design (bf16 operands, fp32 PSUM accumulation):
  - The exp'd relative-position bias diagonals are computed on the HOST
    (structural bucket table x rel_emb gather + exp -> [8, 4096] bf16) and
    DMA'd in; per-head Toeplitz tiles U are built with one sheared DMA each.
  - Q^T is stored with s REVERSED so the bias becomes a positive-shear
    Toeplitz.
  - Phase A: ONE streaming pass over x^T per s-chunk computes Q^T/K^T (head
    pair 0) AND V (all heads) -> PE ~100% busy, x^T read once here.
  - Phase B: per (pair, qc) attention kt-loop; scores via row-packed pair of
    K=64 matmuls (both PE-array halves concurrently); ONE ACT exp per kt
    covers both heads [128,1024] (ACT cadence ~1.2us/iter); DVE multiplies
    by the Toeplitz exp-bias; PV uses V_aug=[V | ones] so psum rows 64:128
    carry the softmax denominator for free.  The NEXT pair's Q/K projection
    matmuls are interleaved one kd-step per kt-iteration (keeps the PE
    queue dense at full p-state); their psum evictions run at the producing
    chunk's tail (deferring them across the chunk boundary races with the
    ring-1 psum slot reuse and must NOT be done).
  - Softmax normalization is deferred: denominator rows -> DRAM, DVE
    reciprocal, broadcast back via DRAM, multiplied into ctxt -- pipelined
    two chunks behind so no engine waits on the DMA round trips.
  - Phase C: output projection immediately after the last attention qc;
    psum evictions alternate ACT/DVE; chunk order puts the last-normalized
    q-range last.
"""

import math
import sys

for _p in ("/opt/trn_rl_repo",):
    if _p not in sys.path:
        sys.path.insert(0, _p)

import numpy as np

import concourse.bass as bass
import concourse.mybir as mybir
import concourse.tile as tile
from concourse import bacc
from concourse.bass_utils import run_bass_kernel_spmd

DT = mybir.dt
AF = mybir.ActivationFunctionType
OP = mybir.AluOpType

# ---- problem constants (hardcoded per contract) ----
B, S, D = 2, 2048, 2048
N_HEADS, D_KV = 32, 64
NUM_BUCKETS, MAX_DISTANCE = 32, 128
NCORES = 8
HL = 8            # heads per core
P = 128
SC = 512          # free-dim chunk
NKT = S // P      # 16 k-tiles
NQC = S // SC     # 4 q-chunks
NDT = D // P      # 16 D-tiles
NMT = (HL * D_KV) // P   # 4 hd m-tiles per core
W_U = 3968        # toeplitz tile width: max j0 (=15*128+3*512=3456) + 512
NDIAG = 4096      # ud row stride


def _rel_bucket_host(d):
    """Exact numpy replica of reference._relative_position_bucket (fp32 math,
    int32 truncation) for bidirectional buckets. d = k - q (int array)."""
    num_buckets = NUM_BUCKETS // 2          # 16
    max_exact = num_buckets // 2            # 8
    rel = np.asarray(d, dtype=np.int64)
    buckets = (rel > 0).astype(np.int32) * num_buckets
    arel = np.abs(rel)
    is_small = arel < max_exact
    rp_safe = np.maximum(arel, 1).astype(np.float32)
    log_ratio = np.log(rp_safe / np.float32(max_exact)).astype(np.float32)
    scale = np.float32(math.log(MAX_DISTANCE / max_exact))
    rp_large = max_exact + (log_ratio / scale * np.float32(num_buckets - max_exact)).astype(np.int32)
    rp_large = np.minimum(rp_large, num_buckets - 1)
    buckets = buckets + np.where(is_small, arel.astype(np.int32), rp_large)
    return buckets.astype(np.int32)


_BUCKETS = _rel_bucket_host(np.arange(NDIAG - 1) - (S - 1))  # structural


def _build(debug=False):
    nc = bacc.Bacc(None, name="attn_tp2")

    xt = nc.declare_dram_parameter("xt", [D, S], DT.bfloat16, isOutput=False)
    wq = nc.declare_dram_parameter("wq", [D, HL * D_KV], DT.bfloat16, isOutput=False)
    wk = nc.declare_dram_parameter("wk", [D, HL * D_KV], DT.bfloat16, isOutput=False)
    wv = nc.declare_dram_parameter("wv", [D, HL * D_KV], DT.bfloat16, isOutput=False)
    wo = nc.declare_dram_parameter("wo", [HL * D_KV, D], DT.bfloat16, isOutput=False)
    mask = nc.declare_dram_parameter("mask", [S], DT.float32, isOutput=False)
    ud = nc.declare_dram_parameter("ud", [HL, NDIAG], DT.bfloat16, isOutput=False)
    out = nc.declare_dram_parameter("out", [S, D], DT.bfloat16, isOutput=True)
    if debug:
        dbg_qt = nc.declare_dram_parameter("dbg_qt", [P, NMT * S], DT.bfloat16, isOutput=True)
        dbg_kt = nc.declare_dram_parameter("dbg_kt", [P, NMT * S], DT.bfloat16, isOutput=True)
        dbg_va = nc.declare_dram_parameter("dbg_va", [P, NKT * HL * 2 * D_KV], DT.bfloat16, isOutput=True)
        dbg_cx = nc.declare_dram_parameter("dbg_cx", [P, NMT * S], DT.bfloat16, isOutput=True)
        dbg_rc = nc.declare_dram_parameter("dbg_rc", [HL * NQC, SC], DT.float32, isOutput=True)

    with tile.TileContext(nc) as tc:
        with (
            tc.tile_pool(name="res", bufs=1) as res,          # persistent tensors
            tc.tile_pool(name="xtp", bufs=8) as xtp,         # x^T stream tiles
            tc.tile_pool(name="stage", bufs=2) as stage,      # staging
            tc.tile_pool(name="upool", bufs=3) as upool,      # toeplitz exp-bias tiles
            tc.tile_pool(name="pexp", bufs=3) as pexpp,       # probs tiles
            tc.tile_pool(name="outp", bufs=5) as outp,        # out staging
            tc.tile_pool(name="psum", bufs=1, space="PSUM") as psum,
            tc.tile_pool(name="dram", bufs=1, space="DRAM") as dramp,
        ):
            # psum tags: "s" ring2 + "cx" ring1 + "pj" ring1, each [128,1024]
            # fp32 (2 banks) -> exactly 8 banks.
            def ps_tile(tag, name):
                return psum.tile([P, 2 * SC], DT.float32, tag=tag, name=name,
                                 bufs=2 if tag == "s" else 1)

            # ---------- constants ----------
            # vaug ones-block init (overlaps the initial weight DMAs)
            vaug = res.tile([P, NKT, HL, 2 * D_KV], DT.bfloat16, tag="vaug")
            nc.vector.memset(vaug[:], 1.0)

            mask_sb = res.tile([P, NKT], DT.float32, tag="mask")
            nc.sync.dma_start(mask_sb[:], mask.ap().rearrange("(kt p) -> p kt", p=P))

            den_dram = dramp.tile([HL * NQC, SC], DT.float32)
            rcp_dram = dramp.tile([HL * NQC, SC], DT.float32)

            # weights (resident, bf16); interleave per-kd chunks with the
            # first s-chunk's x^T tiles so phase A's PE can start within ~2us.
            wq_sb = res.tile([P, NDT, HL * D_KV], DT.bfloat16, tag="wq")
            wk_sb = res.tile([P, NDT, HL * D_KV], DT.bfloat16, tag="wk")
            wv_sb = res.tile([P, NDT, HL * D_KV], DT.bfloat16, tag="wv")
            wo_sb = res.tile([P, NMT, D], DT.bfloat16, tag="wo")
            pre_xt = {}
            for kd in range(NDT):
                if kd < 8:
                    t = xtp.tile([P, SC], DT.bfloat16, tag="xt",
                                 name=f"xa0_{kd}")
                    nc.sync.dma_start(t[:], xt[kd * P:(kd + 1) * P, 0:SC])
                    pre_xt[kd] = t
                # phase A needs only pair 0's columns of Wq/Wk; the rest is
                # for the interleaved projections ~80us later (loaded below)
                nc.sync.dma_start(wq_sb[:, kd, 0:P], wq[kd * P:(kd + 1) * P, 0:P])
                nc.sync.dma_start(wk_sb[:, kd, 0:P], wk[kd * P:(kd + 1) * P, 0:P])
                nc.sync.dma_start(wv_sb[:, kd, :], wv[kd * P:(kd + 1) * P, :])

            # persistent activations
            qt_sb = res.tile([P, NMT, S], DT.bfloat16, tag="qt")   # q REVERSED
            kt_sb = res.tile([P, NMT, S], DT.bfloat16, tag="kt")
            ctxt = res.tile([P, NMT, S], DT.bfloat16, tag="ctxt")

            def load_u(pr):
                u_t = {}
                for hh in (2 * pr, 2 * pr + 1):
                    u = upool.tile([P, W_U], DT.bfloat16, tag="u", name=f"u{hh}")
                    uda = ud.ap()
                    shear = bass.AP(
                        tensor=uda.tensor,
                        offset=uda.offset + hh * NDIAG,
                        ap=[[1, P], [1, W_U]],
                    )
                    nc.sync.dma_start(u[:], shear)
                    u_t[hh] = u
                return u_t

            def rev_ap(base, start_col, total):
                """AP over `base` writing SC columns reversed: column j of the
                source lands at logical position total-1-(start_col+j)."""
                return bass.AP(
                    tensor=base.tensor,
                    offset=base.offset + (total - 1 - start_col),
                    ap=[list(base.ap[0]), [-1, SC]],
                )

            # ---------- phase A: fused pass: Q/K (pair 0) + V (all heads) ----
            for nq in range(NQC):
                qk_ps = ps_tile("s", f"aqk{nq}")
                q_ps, k_ps = qk_ps[:, 0:SC], qk_ps[:, SC:2 * SC]
                v01 = ps_tile("cx", f"av01_{nq}")
                v23 = ps_tile("pj", f"av23_{nq}")
                v_ps = [v01[:, 0:SC], v01[:, SC:2 * SC],
                        v23[:, 0:SC], v23[:, SC:2 * SC]]
                for kd in range(NDT):
                    if nq == 0 and kd in pre_xt:
                        xt_t = pre_xt.pop(kd)
                    else:
                        xt_t = xtp.tile([P, SC], DT.bfloat16, tag="xt",
                                        name=f"xa{nq}_{kd}")
                        nc.sync.dma_start(
                            xt_t[:],
                            xt[kd * P:(kd + 1) * P, nq * SC:(nq + 1) * SC]
                        )
                    nc.tensor.matmul(
                        q_ps, wq_sb[:, kd, 0:P], xt_t[:],
                        start=(kd == 0), stop=(kd == NDT - 1),
                    )
                    nc.tensor.matmul(
                        k_ps, wk_sb[:, kd, 0:P], xt_t[:],
                        start=(kd == 0), stop=(kd == NDT - 1),
                    )
                    for st in range(4):
                        nc.tensor.matmul(
                            v_ps[st], xt_t[:, st * P:(st + 1) * P],
                            wv_sb[:, kd, :],
                            start=(kd == 0), stop=(kd == NDT - 1),
                        )
                # evictions on ACT (idle through all of phase A; DVE
                # evictions made the next chunk's first V matmul wait and
                # cascaded into the xt prefetch ring)
                for st in range(4):
                    kt_glob = nq * 4 + st
                    nc.scalar.copy(
                        vaug[:, kt_glob, :, 0:D_KV],
                        v_ps[st].rearrange("p (h d) -> p h d", d=D_KV),
                    )
                nc.scalar.copy(rev_ap(qt_sb[:, 0, :], nq * SC, S), q_ps)
                nc.scalar.copy(kt_sb[:, 0, nq * SC:(nq + 1) * SC], k_ps)

            # phase-B-only inputs: emitted after ALL of phase A's DMA
            # traffic (anything placed earlier delays nq>=1 xt streams by
            # queueing in front of them).  u tiles first (needed at the very
            # start of phase B), then the projection weight remainders, wo.
            u_t = load_u(0)
            for kd in range(NDT):
                nc.sync.dma_start(wq_sb[:, kd, P:HL * D_KV],
                                  wq[kd * P:(kd + 1) * P, P:HL * D_KV])
                nc.sync.dma_start(wk_sb[:, kd, P:HL * D_KV],
                                  wk[kd * P:(kd + 1) * P, P:HL * D_KV])
            for mt in range(NMT):
                nc.sync.dma_start(wo_sb[:, mt, :],
                                  wo[mt * P:(mt + 1) * P, :])

            # ---------- phase B ----------
            def attn_qc(pr, qc, u_t, proj_pr, pnq, dve_slots):
                """Attention for head pair pr, reversed-col chunk qc.
                proj_pr/pnq: pair + s-chunk whose Q/K projection kd-steps
                interleave here (starting at kt=3).  dve_slots: kt -> list of
                thunks emitting deferred non-critical DVE work right after
                that iteration's critical multiplies."""
                h0, h1 = 2 * pr, 2 * pr + 1
                jg0 = qc * SC
                cx01 = ps_tile("cx", f"cx{pr}_{qc}")
                cx0, cx1 = cx01[:, 0:SC], cx01[:, SC:2 * SC]
                if proj_pr is not None:
                    jp0 = pnq * SC
                    pj_ps = ps_tile("pj", f"pj{proj_pr}_{pnq}")
                    pq_ps, pk_ps = pj_ps[:, 0:SC], pj_ps[:, SC:2 * SC]
                    pxt = {}
                    for kd in range(2):
                        t = xtp.tile([P, SC], DT.bfloat16, tag="xt",
                                     name=f"xp{proj_pr}_{pnq}_{kd}")
                        nc.sync.dma_start(
                            t[:], xt[kd * P:(kd + 1) * P, jp0:jp0 + SC])
                        pxt[kd] = t
                    # kd steps per kt iteration (start at 2, catch up at end):
                    # kt 2..13 -> kd 0..11 ; 14 -> 12,13 ; 15 -> 14,15
                    kd_sched = {kt: [kt - 2] for kt in range(2, NKT - 2)}
                    kd_sched[NKT - 2] = [NKT - 4, NKT - 3]
                    kd_sched[NKT - 1] = [NKT - 2, NKT - 1]

                def emit_qk(kt):
                    s01 = ps_tile("s", f"s{pr}_{qc}_{kt}")
                    nc.tensor.matmul(
                        s01[:, 0:SC], kt_sb[0:64, pr, kt * P:(kt + 1) * P],
                        qt_sb[0:64, pr, jg0:jg0 + SC],
                        start=True, stop=True, tile_position=(0, 0),
                    )
                    nc.tensor.matmul(
                        s01[:, SC:2 * SC], kt_sb[64:128, pr, kt * P:(kt + 1) * P],
                        qt_sb[64:128, pr, jg0:jg0 + SC],
                        start=True, stop=True, tile_position=(64, 0),
                    )
                    return s01

                s01 = emit_qk(0)
                for kt in range(NKT):
                    s01_next = emit_qk(kt + 1) if kt + 1 < NKT else None
                    px = pexpp.tile([P, 2 * SC], DT.bfloat16, tag="pexp",
                                    name=f"px{pr}_{qc}_{kt}")
                    nc.scalar.activation(
                        out=px[:], in_=s01[:], func=AF.Exp,
                        bias=mask_sb[:, kt:kt + 1], scale=1.0 / math.sqrt(D_KV),
                    )
                    j0 = kt * P + jg0
                    nc.vector.tensor_tensor(
                        px[:, 0:SC], px[:, 0:SC], u_t[h0][:, j0:j0 + SC], OP.mult
                    )
                    nc.vector.tensor_tensor(
                        px[:, SC:2 * SC], px[:, SC:2 * SC],
                        u_t[h1][:, j0:j0 + SC], OP.mult
                    )
                    for thunk in dve_slots.get(kt, ()):
                        thunk()
                    if proj_pr is not None:
                        for kd in kd_sched.get(kt, ()):
                            nc.tensor.matmul(
                                pq_ps, wq_sb[:, kd, proj_pr * P:(proj_pr + 1) * P],
                                pxt[kd][:],
                                start=(kd == 0), stop=(kd == NDT - 1),
                            )
                            nc.tensor.matmul(
                                pk_ps, wk_sb[:, kd, proj_pr * P:(proj_pr + 1) * P],
                                pxt[kd][:],
                                start=(kd == 0), stop=(kd == NDT - 1),
                            )
                            del pxt[kd]
                            nkd = kd + 2
                            if nkd < NDT:
                                t = xtp.tile([P, SC], DT.bfloat16, tag="xt",
                                             name=f"xp{proj_pr}_{pnq}_{nkd}")
                                nc.sync.dma_start(
                                    t[:],
                                    xt[nkd * P:(nkd + 1) * P, jp0:jp0 + SC])
                                pxt[nkd] = t
                    nc.tensor.matmul(
                        cx0, vaug[:, kt, h0, :], px[:, 0:SC],
                        start=(kt == 0), stop=(kt == NKT - 1),
                    )
                    nc.tensor.matmul(
                        cx1, vaug[:, kt, h1, :], px[:, SC:2 * SC],
                        start=(kt == 0), stop=(kt == NKT - 1),
                    )
                    s01 = s01_next

                # ctx eviction (unnormalized, un-reversing q) + denominator
                # rows un-reversed into SBUF; reciprocal + DRAM broadcast is
                # deferred to the NEXT chunk's DVE slots.
                dns = []
                for hh, cx in ((h0, cx0), (h1, cx1)):
                    off = (hh % 2) * 64
                    base = ctxt[off:off + 64, pr, :]
                    nc.vector.tensor_copy(rev_ap(base, jg0, S), cx[0:D_KV, :])
                    dn = stage.tile([P, SC], DT.float32, tag="dn",
                                    name=f"dn{hh}_{qc}", bufs=2)
                    nc.vector.tensor_copy(
                        rev_ap(dn[64:65, :], 0, SC), cx[64:65, :])
                    nc.sync.dma_start(den_dram[hh * NQC + qc, :], dn[64:65, :])
                    dns.append(dn)

                # projection psum evictions after the cx evictions (cx slot
                # is needed by the next chunk's first PV; the pj slot only by
                # its kt=2 projection matmul).  Same-chunk emission: deferring
                # these across the boundary races with ring-1 slot reuse.
                if proj_pr is not None:
                    nc.vector.tensor_copy(
                        rev_ap(qt_sb[:, proj_pr, :], jp0, S), pq_ps)
                    nc.vector.tensor_copy(
                        kt_sb[:, proj_pr, jp0:jp0 + SC], pk_ps)

                return None, None

            def norm_fetch(pr, qc):
                rows = [2 * pr * NQC + qc, (2 * pr + 1) * NQC + qc]
                den2 = stage.tile([2, SC], DT.float32, tag="den2",
                                  name=f"de{pr}_{qc}", bufs=1)
                for r, row in enumerate(rows):
                    nc.sync.dma_start(den2[r:r + 1, :], den_dram[row, :])
                return {"pr": pr, "qc": qc, "rows": rows, "den2": den2}

            def norm_rcp(rec):
                rcp2 = stage.tile([2, SC], DT.float32, tag="rcp2",
                                  name=f"rc{rec['pr']}_{rec['qc']}", bufs=1)
                nc.vector.reciprocal_approx_fast(
                    out=rcp2[:], in_=rec["den2"][:])
                rbs = []
                for r, row in enumerate(rec["rows"]):
                    nc.sync.dma_start(rcp_dram[row, :], rcp2[r:r + 1, :])
                    off = r * 64
                    rb = stage.tile([P, SC], DT.float32, tag="rb",
                                    name=f"rb{rec['pr']}_{rec['qc']}_{r}", bufs=4)
                    bcast = bass.AP(
                        tensor=rcp_dram.tensor,
                        offset=rcp_dram.offset + row * SC,
                        ap=[[0, D_KV], [1, SC]],
                    )
                    nc.sync.dma_start(rb[off:off + D_KV, :], bcast)
                    rbs.append(rb)
                rec["rbs"] = rbs

            def norm_apply(rec):
                """Multiply ctxt rows of (pr, qc) by broadcast reciprocals."""
                q0t = S - (rec["qc"] + 1) * SC
                for r in range(2):
                    off = r * 64
                    cslc = ctxt[off:off + 64, rec["pr"], q0t:q0t + SC]
                    nc.vector.tensor_tensor(
                        cslc, cslc, rec["rbs"][r][off:off + D_KV, :], OP.mult)

            nrecs = []
            prev_ev = None
            for pr in range(HL // 2):
                proj_pr = pr + 1 if pr + 1 < HL // 2 else None
                if proj_pr is not None:
                    next_u = load_u(proj_pr)
                for qc in range(NQC):
                    prev_ev, rec = attn_qc(pr, qc, u_t, proj_pr, qc, {})
                    nrecs.append(norm_fetch(pr, qc))
                    if len(nrecs) >= 2:
                        norm_rcp(nrecs[-2])
                    if len(nrecs) >= 3:
                        norm_apply(nrecs[-3])
                if proj_pr is not None:
                    u_t = next_u
            norm_rcp(nrecs[-1])
            norm_apply(nrecs[-2])
            norm_apply(nrecs[-1])  # rb DMA lands a few us into phase C

            # ---------- phase C: output projection ----------
            # ctxt columns were normalized in order qc=0..3 i.e. column
            # blocks [1536,2048), [1024,1536), ... -> last-normalized last.
            st_order = []
            for qc in range(NQC):
                q0t = S - (qc + 1) * SC
                st_order.extend(range(q0t // P, q0t // P + NQC))
            tags = ["s", "s", "cx", "pj"]
            ti = 0
            for st in st_order:
                for ndp in range(2):  # two [128,1024] psum tiles per st
                    o2 = ps_tile(tags[ti % 4], f"o{st}_{ndp}")
                    for half in range(2):
                        nd = 2 * ndp + half
                        o_ps = o2[:, half * SC:(half + 1) * SC]
                        for m in range(NMT):
                            nc.tensor.matmul(
                                o_ps, ctxt[:, m, st * P:(st + 1) * P],
                                wo_sb[:, m, nd * SC:(nd + 1) * SC],
                                start=(m == 0), stop=(m == NMT - 1),
                            )
                    # evict halves on alternating engines
                    for half in range(2):
                        nd = 2 * ndp + half
                        o_t = outp.tile([P, SC], DT.bfloat16, tag="out",
                                        name=f"ot{st}_{nd}")
                        if (ti + half) % 2 == 0:
                            nc.scalar.copy(o_t[:], o2[:, half * SC:(half + 1) * SC])
                        else:
                            nc.vector.tensor_copy(
                                o_t[:], o2[:, half * SC:(half + 1) * SC])
                        nc.sync.dma_start(
                            out[st * P:(st + 1) * P, nd * SC:(nd + 1) * SC],
                            o_t[:])
                    ti += 1

    nc.finalize()
    return nc


_NC_CACHE = None


def _get_nc():
    global _NC_CACHE
    if _NC_CACHE is None:
        _NC_CACHE = _build()
    return _NC_CACHE


def _in_maps(hidden_states, attention_mask, Wq, Wk, Wv, Wo, rel_emb):
    import ml_dtypes
    bf16 = ml_dtypes.bfloat16
    maps = []
    for c in range(NCORES):
        b, g = c // 4, c % 4
        hlo, hhi = g * HL, (g + 1) * HL
        udm = np.zeros((HL, NDIAG), dtype=np.float32)
        udm[:, :NDIAG - 1] = np.exp(rel_emb[_BUCKETS, hlo:hhi]).T
        maps.append({
            "xt": np.ascontiguousarray(hidden_states[b].T).astype(bf16),
            "wq": np.ascontiguousarray(Wq[:, hlo * D_KV:hhi * D_KV]).astype(bf16),
            "wk": np.ascontiguousarray(Wk[:, hlo * D_KV:hhi * D_KV]).astype(bf16),
            "wv": np.ascontiguousarray(Wv[:, hlo * D_KV:hhi * D_KV]).astype(bf16),
            "wo": np.ascontiguousarray(Wo[hlo * D_KV:hhi * D_KV, :]).astype(bf16),
            "mask": np.ascontiguousarray(attention_mask[b, 0, 0, :]).astype(np.float32),
            "ud": udm.astype(bf16),
        })
    return maps


def kernel(hidden_states, attention_mask, Wq, Wk, Wv, Wo, rel_emb, _trace=False,
           _trace_kwargs=None):
    hidden_states = np.asarray(hidden_states, dtype=np.float32)
    attention_mask = np.asarray(attention_mask, dtype=np.float32)
    Wq = np.asarray(Wq, dtype=np.float32)
    Wk = np.asarray(Wk, dtype=np.float32)
    Wv = np.asarray(Wv, dtype=np.float32)
    Wo = np.asarray(Wo, dtype=np.float32)
    rel_emb = np.asarray(rel_emb, dtype=np.float32)

    nc = _get_nc()
    maps = _in_maps(hidden_states, attention_mask, Wq, Wk, Wv, Wo, rel_emb)
    kw = dict(_trace_kwargs or {})
    res = run_bass_kernel_spmd(nc, maps, core_ids=list(range(NCORES)),
                               trace=_trace, **kw)
    kernel.last_results = res
    outp = np.empty((B, S, D), dtype=np.float32)
    for b in range(B):
        acc = np.asarray(res.results[4 * b]["out"], dtype=np.float32).copy()
        for g in range(1, 4):
            acc += np.asarray(res.results[4 * b + g]["out"], dtype=np.float32)
        outp[b] = acc
    return outp


# revision 45
# speedup vs baseline: 1.2015x; 1.0062x over previous
"""T5-style encoder self-attention (dense_transformer) on 8 Trainium2 NeuronCores.

Problem (full shapes): hidden [2,2048,2048], Wq/Wk/Wv/Wo [2048,2048],
rel_emb [32,32] (bidirectional T5 relative-position bias), mask [2,1,1,2048].

Sharding: data-parallel over batch (2) x tensor-parallel over heads (4 groups
of 8 heads) = 8 cores, Megatron-style. Each core computes a partial output
[2048,2048] for its batch (its 8 heads through its Wo row-slice); the host
sums 4 partials per batch.

Per-core kernel design (bf16 operands, fp32 PSUM accumulation):
  - The exp'd relative-position bias diagonals are computed on the HOST
    (structural bucket table x rel_emb gather + exp -> [8, 4096] bf16) and
    DMA'd in; per-head Toeplitz tiles U are built with one sheared DMA each.
  - Q^T is stored with s REVERSED so the bias becomes a positive-shear
    Toeplitz.
  - Phase A: ONE streaming pass over x^T per s-chunk computes Q^T/K^T (head
    pair 0) AND V (all heads) -> PE ~100% busy, x^T read once here.
  - Phase B: per (pair, qc) attention kt-loop; scores via row-packed pair of
    K=64 matmuls (both PE-array halves concurrently); ONE ACT exp per kt
    covers both heads [128,1024] (ACT cadence ~1.2us/iter); DVE multiplies
    by the Toeplitz exp-bias; PV uses V_aug=[V | ones] so psum rows 64:128
    carry the softmax denominator for free.  The NEXT pair's Q/K projection
    matmuls are interleaved one kd-step per kt-iteration (keeps the PE
    queue dense at full p-state); their psum evictions run at the producing
    chunk's tail (deferring them across the chunk boundary races with the
    ring-1 psum slot reuse and must NOT be done).
  - Softmax normalization is deferred: denominator rows -> DRAM, DVE
    reciprocal, broadcast back via DRAM, multiplied into ctxt -- pipelined
    two chunks behind so no engine waits on the DMA round trips.
  - Phase C: output projection immediately after the last attention qc;
    psum evictions alternate ACT/DVE; chunk order puts the last-normalized
    q-range last.
"""

import math
import sys

for _p in ("/opt/trn_rl_repo",):
    if _p not in sys.path:
        sys.path.insert(0, _p)

import numpy as np

import concourse.bass as bass
import concourse.mybir as mybir
import concourse.tile as tile
from concourse import bacc
from concourse.bass_utils import run_bass_kernel_spmd

DT = mybir.dt
AF = mybir.ActivationFunctionType
OP = mybir.AluOpType

# ---- problem constants (hardcoded per contract) ----
B, S, D = 2, 2048, 2048
N_HEADS, D_KV = 32, 64
NUM_BUCKETS, MAX_DISTANCE = 32, 128
NCORES = 8
HL = 8            # heads per core
P = 128
SC = 512          # free-dim chunk
NKT = S // P      # 16 k-tiles
NQC = S // SC     # 4 q-chunks
NDT = D // P      # 16 D-tiles
NMT = (HL * D_KV) // P   # 4 hd m-tiles per core
W_U = 3968        # toeplitz tile width: max j0 (=15*128+3*512=3456) + 512
NDIAG = 4096      # ud row stride


def _rel_bucket_host(d):
    """Exact numpy replica of reference._relative_position_bucket (fp32 math,
    int32 truncation) for bidirectional buckets. d = k - q (int array)."""
    num_buckets = NUM_BUCKETS // 2          # 16
    max_exact = num_buckets // 2            # 8
    rel = np.asarray(d, dtype=np.int64)
    buckets = (rel > 0).astype(np.int32) * num_buckets
    arel = np.abs(rel)
    is_small = arel < max_exact
    rp_safe = np.maximum(arel, 1).astype(np.float32)
    log_ratio = np.log(rp_safe / np.float32(max_exact)).astype(np.float32)
    scale = np.float32(math.log(MAX_DISTANCE / max_exact))
    rp_large = max_exact + (log_ratio / scale * np.float32(num_buckets - max_exact)).astype(np.int32)
    rp_large = np.minimum(rp_large, num_buckets - 1)
    buckets = buckets + np.where(is_small, arel.astype(np.int32), rp_large)
    return buckets.astype(np.int32)


_BUCKETS = _rel_bucket_host(np.arange(NDIAG - 1) - (S - 1))  # structural


def _build(debug=False):
    nc = bacc.Bacc(None, name="attn_tp2")

    xt = nc.declare_dram_parameter("xt", [D, S], DT.bfloat16, isOutput=False)
    wq = nc.declare_dram_parameter("wq", [D, HL * D_KV], DT.bfloat16, isOutput=False)
    wk = nc.declare_dram_parameter("wk", [D, HL * D_KV], DT.bfloat16, isOutput=False)
    wv = nc.declare_dram_parameter("wv", [D, HL * D_KV], DT.bfloat16, isOutput=False)
    wo = nc.declare_dram_parameter("wo", [HL * D_KV, D], DT.bfloat16, isOutput=False)
    mask = nc.declare_dram_parameter("mask", [S], DT.float32, isOutput=False)
    ud = nc.declare_dram_parameter("ud", [HL, NDIAG], DT.bfloat16, isOutput=False)
    out = nc.declare_dram_parameter("out", [S, D], DT.bfloat16, isOutput=True)
    if debug:
        dbg_qt = nc.declare_dram_parameter("dbg_qt", [P, NMT * S], DT.bfloat16, isOutput=True)
        dbg_kt = nc.declare_dram_parameter("dbg_kt", [P, NMT * S], DT.bfloat16, isOutput=True)
        dbg_va = nc.declare_dram_parameter("dbg_va", [P, NKT * HL * 2 * D_KV], DT.bfloat16, isOutput=True)
        dbg_cx = nc.declare_dram_parameter("dbg_cx", [P, NMT * S], DT.bfloat16, isOutput=True)
        dbg_rc = nc.declare_dram_parameter("dbg_rc", [HL * NQC, SC], DT.float32, isOutput=True)

    with tile.TileContext(nc) as tc:
        with (
            tc.tile_pool(name="res", bufs=1) as res,          # persistent tensors
            tc.tile_pool(name="xtp", bufs=8) as xtp,         # x^T stream tiles
            tc.tile_pool(name="stage", bufs=2) as stage,      # staging
            tc.tile_pool(name="upool", bufs=3) as upool,      # toeplitz exp-bias tiles
            tc.tile_pool(name="pexp", bufs=4) as pexpp,       # probs tiles
            tc.tile_pool(name="outp", bufs=5) as outp,        # out staging
            tc.tile_pool(name="psum", bufs=1, space="PSUM") as psum,
            tc.tile_pool(name="dram", bufs=1, space="DRAM") as dramp,
        ):
            # psum tags: "s" ring2 + "cx" ring1 + "pj" ring1, each [128,1024]
            # fp32 (2 banks) -> exactly 8 banks.
            def ps_tile(tag, name):
                return psum.tile([P, 2 * SC], DT.float32, tag=tag, name=name,
                                 bufs=2 if tag == "s" else 1)

            # ---------- constants ----------
            # vaug ones-block init (overlaps the initial weight DMAs)
            vaug = res.tile([P, NKT, HL, 2 * D_KV], DT.bfloat16, tag="vaug")
            nc.vector.memset(vaug[:], 1.0)

            mask_sb = res.tile([P, NKT], DT.float32, tag="mask")
            nc.sync.dma_start(mask_sb[:], mask.ap().rearrange("(kt p) -> p kt", p=P))

            den_dram = dramp.tile([HL * NQC, SC], DT.float32)
            rcp_dram = dramp.tile([HL * NQC, SC], DT.float32)

            # weights (resident, bf16); interleave per-kd chunks with the
            # first s-chunk's x^T tiles so phase A's PE can start within ~2us.
            wq_sb = res.tile([P, NDT, HL * D_KV], DT.bfloat16, tag="wq")
            wk_sb = res.tile([P, NDT, HL * D_KV], DT.bfloat16, tag="wk")
            wv_sb = res.tile([P, NDT, HL * D_KV], DT.bfloat16, tag="wv")
            wo_sb = res.tile([P, NMT, D], DT.bfloat16, tag="wo")
            pre_xt = {}
            for kd in range(NDT):
                if kd < 8:
                    t = xtp.tile([P, SC], DT.bfloat16, tag="xt",
                                 name=f"xa0_{kd}")
                    nc.sync.dma_start(t[:], xt[kd * P:(kd + 1) * P, 0:SC])
                    pre_xt[kd] = t
                # phase A needs only pair 0's columns of Wq/Wk; the rest is
                # for the interleaved projections ~80us later (loaded below)
                nc.sync.dma_start(wq_sb[:, kd, 0:P], wq[kd * P:(kd + 1) * P, 0:P])
                nc.sync.dma_start(wk_sb[:, kd, 0:P], wk[kd * P:(kd + 1) * P, 0:P])
                nc.sync.dma_start(wv_sb[:, kd, :], wv[kd * P:(kd + 1) * P, :])

            # persistent activations
            qt_sb = res.tile([P, NMT, S], DT.bfloat16, tag="qt")   # q REVERSED
            kt_sb = res.tile([P, NMT, S], DT.bfloat16, tag="kt")
            ctxt = res.tile([P, NMT, S], DT.bfloat16, tag="ctxt")

            def load_u(pr):
                u_t = {}
                for hh in (2 * pr, 2 * pr + 1):
                    u = upool.tile([P, W_U], DT.bfloat16, tag="u", name=f"u{hh}")
                    uda = ud.ap()
                    shear = bass.AP(
                        tensor=uda.tensor,
                        offset=uda.offset + hh * NDIAG,
                        ap=[[1, P], [1, W_U]],
                    )
                    nc.sync.dma_start(u[:], shear)
                    u_t[hh] = u
                return u_t

            def rev_ap(base, start_col, total):
                """AP over `base` writing SC columns reversed: column j of the
                source lands at logical position total-1-(start_col+j)."""
                return bass.AP(
                    tensor=base.tensor,
                    offset=base.offset + (total - 1 - start_col),
                    ap=[list(base.ap[0]), [-1, SC]],
                )

            # ---------- phase A: fused pass: Q/K (pair 0) + V (all heads) ----
            for nq in range(NQC):
                qk_ps = ps_tile("s", f"aqk{nq}")
                q_ps, k_ps = qk_ps[:, 0:SC], qk_ps[:, SC:2 * SC]
                v01 = ps_tile("cx", f"av01_{nq}")
                v23 = ps_tile("pj", f"av23_{nq}")
                v_ps = [v01[:, 0:SC], v01[:, SC:2 * SC],
                        v23[:, 0:SC], v23[:, SC:2 * SC]]
                # V matmuls lag Q/K by VLAG kd-steps: at the chunk
                # boundary the first V matmul waits on the previous chunk's
                # V-psum evictions -- the lag keeps Q/K work ahead of it in
                # the in-order PE queue instead of stalling head-of-queue.
                VLAG = 4
                live_xt = {}

                def emit_v(kdv):
                    xv = live_xt.pop(kdv)
                    for st in range(4):
                        nc.tensor.matmul(
                            v_ps[st], xv[:, st * P:(st + 1) * P],
                            wv_sb[:, kdv, :],
                            start=(kdv == 0), stop=(kdv == NDT - 1),
                        )

                for kd in range(NDT):
                    if nq == 0 and kd in pre_xt:
                        xt_t = pre_xt.pop(kd)
                    else:
                        xt_t = xtp.tile([P, SC], DT.bfloat16, tag="xt",
                                        name=f"xa{nq}_{kd}")
                        nc.sync.dma_start(
                            xt_t[:],
                            xt[kd * P:(kd + 1) * P, nq * SC:(nq + 1) * SC]
                        )
                    live_xt[kd] = xt_t
                    nc.tensor.matmul(
                        q_ps, wq_sb[:, kd, 0:P], xt_t[:],
                        start=(kd == 0), stop=(kd == NDT - 1),
                    )
                    nc.tensor.matmul(
                        k_ps, wk_sb[:, kd, 0:P], xt_t[:],
                        start=(kd == 0), stop=(kd == NDT - 1),
                    )
                    if kd >= VLAG:
                        emit_v(kd - VLAG)
                for kdv in range(NDT - VLAG, NDT):
                    emit_v(kdv)
                # evictions: v0/v1 first (their psum slots are needed soonest)
                for st in range(4):
                    kt_glob = nq * 4 + st
                    nc.vector.tensor_copy(
                        vaug[:, kt_glob, :, 0:D_KV],
                        v_ps[st].rearrange("p (h d) -> p h d", d=D_KV),
                    )
                nc.vector.tensor_copy(rev_ap(qt_sb[:, 0, :], nq * SC, S), q_ps)
                nc.vector.tensor_copy(kt_sb[:, 0, nq * SC:(nq + 1) * SC], k_ps)

            # phase-B-only inputs: emitted after ALL of phase A's DMA
            # traffic (anything placed earlier delays nq>=1 xt streams by
            # queueing in front of them).  u tiles first (needed at the very
            # start of phase B), then the projection weight remainders, wo.
            u_t = load_u(0)
            for kd in range(NDT):
                nc.sync.dma_start(wq_sb[:, kd, P:HL * D_KV],
                                  wq[kd * P:(kd + 1) * P, P:HL * D_KV])
                nc.sync.dma_start(wk_sb[:, kd, P:HL * D_KV],
                                  wk[kd * P:(kd + 1) * P, P:HL * D_KV])
            for mt in range(NMT):
                nc.sync.dma_start(wo_sb[:, mt, :],
                                  wo[mt * P:(mt + 1) * P, :])

            # ---------- phase B ----------
            def attn_qc(pr, qc, u_t, proj_pr, pnq, dve_slots):
                """Attention for head pair pr, reversed-col chunk qc.
                proj_pr/pnq: pair + s-chunk whose Q/K projection kd-steps
                interleave here (starting at kt=3).  dve_slots: kt -> list of
                thunks emitting deferred non-critical DVE work right after
                that iteration's critical multiplies."""
                h0, h1 = 2 * pr, 2 * pr + 1
                jg0 = qc * SC
                cx01 = ps_tile("cx", f"cx{pr}_{qc}")
                cx0, cx1 = cx01[:, 0:SC], cx01[:, SC:2 * SC]
                if proj_pr is not None:
                    jp0 = pnq * SC
                    pj_ps = ps_tile("pj", f"pj{proj_pr}_{pnq}")
                    pq_ps, pk_ps = pj_ps[:, 0:SC], pj_ps[:, SC:2 * SC]
                    pxt = {}
                    for kd in range(2):
                        t = xtp.tile([P, SC], DT.bfloat16, tag="xt",
                                     name=f"xp{proj_pr}_{pnq}_{kd}")
                        nc.sync.dma_start(
                            t[:], xt[kd * P:(kd + 1) * P, jp0:jp0 + SC])
                        pxt[kd] = t
                    # kd steps per kt iteration (start at 2, catch up at end):
                    # kt 2..13 -> kd 0..11 ; 14 -> 12,13 ; 15 -> 14,15
                    kd_sched = {kt: [kt - 2] for kt in range(2, NKT - 2)}
                    kd_sched[NKT - 2] = [NKT - 4, NKT - 3]
                    kd_sched[NKT - 1] = [NKT - 2, NKT - 1]

                def emit_qk(kt):
                    s01 = ps_tile("s", f"s{pr}_{qc}_{kt}")
                    nc.tensor.matmul(
                        s01[:, 0:SC], kt_sb[0:64, pr, kt * P:(kt + 1) * P],
                        qt_sb[0:64, pr, jg0:jg0 + SC],
                        start=True, stop=True, tile_position=(0, 0),
                    )
                    nc.tensor.matmul(
                        s01[:, SC:2 * SC], kt_sb[64:128, pr, kt * P:(kt + 1) * P],
                        qt_sb[64:128, pr, jg0:jg0 + SC],
                        start=True, stop=True, tile_position=(64, 0),
                    )
                    return s01

                s01 = emit_qk(0)
                for kt in range(NKT):
                    s01_next = emit_qk(kt + 1) if kt + 1 < NKT else None
                    px = pexpp.tile([P, 2 * SC], DT.bfloat16, tag="pexp",
                                    name=f"px{pr}_{qc}_{kt}")
                    nc.scalar.activation(
                        out=px[:], in_=s01[:], func=AF.Exp,
                        bias=mask_sb[:, kt:kt + 1], scale=1.0 / math.sqrt(D_KV),
                    )
                    j0 = kt * P + jg0
                    nc.vector.tensor_tensor(
                        px[:, 0:SC], px[:, 0:SC], u_t[h0][:, j0:j0 + SC], OP.mult
                    )
                    nc.vector.tensor_tensor(
                        px[:, SC:2 * SC], px[:, SC:2 * SC],
                        u_t[h1][:, j0:j0 + SC], OP.mult
                    )
                    for thunk in dve_slots.get(kt, ()):
                        thunk()
                    if proj_pr is not None:
                        for kd in kd_sched.get(kt, ()):
                            nc.tensor.matmul(
                                pq_ps, wq_sb[:, kd, proj_pr * P:(proj_pr + 1) * P],
                                pxt[kd][:],
                                start=(kd == 0), stop=(kd == NDT - 1),
                            )
                            nc.tensor.matmul(
                                pk_ps, wk_sb[:, kd, proj_pr * P:(proj_pr + 1) * P],
                                pxt[kd][:],
                                start=(kd == 0), stop=(kd == NDT - 1),
                            )
                            del pxt[kd]
                            nkd = kd + 2
                            if nkd < NDT:
                                t = xtp.tile([P, SC], DT.bfloat16, tag="xt",
                                             name=f"xp{proj_pr}_{pnq}_{nkd}")
                                nc.sync.dma_start(
                                    t[:],
                                    xt[nkd * P:(nkd + 1) * P, jp0:jp0 + SC])
                                pxt[nkd] = t
                    nc.tensor.matmul(
                        cx0, vaug[:, kt, h0, :], px[:, 0:SC],
                        start=(kt == 0), stop=(kt == NKT - 1),
                    )
                    nc.tensor.matmul(
                        cx1, vaug[:, kt, h1, :], px[:, SC:2 * SC],
                        start=(kt == 0), stop=(kt == NKT - 1),
                    )
                    s01 = s01_next

                # ctx eviction (unnormalized, un-reversing q) + denominator
                # rows un-reversed into SBUF; reciprocal + DRAM broadcast is
                # deferred to the NEXT chunk's DVE slots.
                dns = []
                for hh, cx in ((h0, cx0), (h1, cx1)):
                    off = (hh % 2) * 64
                    base = ctxt[off:off + 64, pr, :]
                    nc.vector.tensor_copy(rev_ap(base, jg0, S), cx[0:D_KV, :])
                    dn = stage.tile([P, SC], DT.float32, tag="dn",
                                    name=f"dn{hh}_{qc}", bufs=2)
                    nc.vector.tensor_copy(
                        rev_ap(dn[64:65, :], 0, SC), cx[64:65, :])
                    nc.sync.dma_start(den_dram[hh * NQC + qc, :], dn[64:65, :])
                    dns.append(dn)

                # projection psum evictions after the cx evictions (cx slot
                # is needed by the next chunk's first PV; the pj slot only by
                # its kt=2 projection matmul).  Same-chunk emission: deferring
                # these across the boundary races with ring-1 slot reuse.
                if proj_pr is not None:
                    nc.vector.tensor_copy(
                        rev_ap(qt_sb[:, proj_pr, :], jp0, S), pq_ps)
                    nc.vector.tensor_copy(
                        kt_sb[:, proj_pr, jp0:jp0 + SC], pk_ps)

                return None, None

            def norm_fetch(pr, qc):
                rows = [2 * pr * NQC + qc, (2 * pr + 1) * NQC + qc]
                den2 = stage.tile([2, SC], DT.float32, tag="den2",
                                  name=f"de{pr}_{qc}", bufs=1)
                for r, row in enumerate(rows):
                    nc.sync.dma_start(den2[r:r + 1, :], den_dram[row, :])
                return {"pr": pr, "qc": qc, "rows": rows, "den2": den2}

            def norm_rcp(rec):
                rcp2 = stage.tile([2, SC], DT.float32, tag="rcp2",
                                  name=f"rc{rec['pr']}_{rec['qc']}", bufs=1)
                nc.vector.reciprocal_approx_fast(
                    out=rcp2[:], in_=rec["den2"][:])
                rbs = []
                for r, row in enumerate(rec["rows"]):
                    nc.sync.dma_start(rcp_dram[row, :], rcp2[r:r + 1, :])
                    off = r * 64
                    rb = stage.tile([P, SC], DT.float32, tag="rb",
                                    name=f"rb{rec['pr']}_{rec['qc']}_{r}", bufs=4)
                    bcast = bass.AP(
                        tensor=rcp_dram.tensor,
                        offset=rcp_dram.offset + row * SC,
                        ap=[[0, D_KV], [1, SC]],
                    )
                    nc.sync.dma_start(rb[off:off + D_KV, :], bcast)
                    rbs.append(rb)
                rec["rbs"] = rbs

            def norm_apply(rec):
                """Multiply ctxt rows of (pr, qc) by broadcast reciprocals."""
                q0t = S - (rec["qc"] + 1) * SC
                for r in range(2):
                    off = r * 64
                    cslc = ctxt[off:off + 64, rec["pr"], q0t:q0t + SC]
                    nc.vector.tensor_tensor(
                        cslc, cslc, rec["rbs"][r][off:off + D_KV, :], OP.mult)

            nrecs = []
            prev_ev = None
            for pr in range(HL // 2):
                proj_pr = pr + 1 if pr + 1 < HL // 2 else None
                if proj_pr is not None:
                    next_u = load_u(proj_pr)
                for qc in range(NQC):
                    prev_ev, rec = attn_qc(pr, qc, u_t, proj_pr, qc, {})
                    nrecs.append(norm_fetch(pr, qc))
                    if len(nrecs) >= 2:
                        norm_rcp(nrecs[-2])
                    if len(nrecs) >= 3:
                        norm_apply(nrecs[-3])
                if proj_pr is not None:
                    u_t = next_u
            norm_rcp(nrecs[-1])
            norm_apply(nrecs[-2])
            norm_apply(nrecs[-1])  # rb DMA lands a few us into phase C

            # ---------- phase C: output projection ----------
            # ctxt columns were normalized in order qc=0..3 i.e. column
            # blocks [1536,2048), [1024,1536), ... -> last-normalized last.
            st_order = []
            for qc in range(NQC):
                q0t = S - (qc + 1) * SC
                st_order.extend(range(q0t // P, q0t // P + NQC))
            tags = ["s", "s", "cx", "pj"]
            ti = 0
            for st in st_order:
                for ndp in range(2):  # two [128,1024] psum tiles per st
                    o2 = ps_tile(tags[ti % 4], f"o{st}_{ndp}")
                    for half in range(2):
                        nd = 2 * ndp + half
                        o_ps = o2[:, half * SC:(half + 1) * SC]
                        for m in range(NMT):
                            nc.tensor.matmul(
                                o_ps, ctxt[:, m, st * P:(st + 1) * P],
                                wo_sb[:, m, nd * SC:(nd + 1) * SC],
                                start=(m == 0), stop=(m == NMT - 1),
                            )
                    # evict halves on alternating engines
                    for half in range(2):
                        nd = 2 * ndp + half
                        o_t = outp.tile([P, SC], DT.bfloat16, tag="out",
                                        name=f"ot{st}_{nd}")
                        if (ti + half) % 2 == 0:
                            nc.scalar.copy(o_t[:], o2[:, half * SC:(half + 1) * SC])
                        else:
                            nc.vector.tensor_copy(
                                o_t[:], o2[:, half * SC:(half + 1) * SC])
                        nc.sync.dma_start(
                            out[st * P:(st + 1) * P, nd * SC:(nd + 1) * SC],
                            o_t[:])
                    ti += 1

    nc.finalize()
    return nc


_NC_CACHE = None


def _get_nc():
    global _NC_CACHE
    if _NC_CACHE is None:
        _NC_CACHE = _build()
    return _NC_CACHE


def _in_maps(hidden_states, attention_mask, Wq, Wk, Wv, Wo, rel_emb):
    import ml_dtypes
    bf16 = ml_dtypes.bfloat16
    maps = []
    for c in range(NCORES):
        b, g = c // 4, c % 4
        hlo, hhi = g * HL, (g + 1) * HL
        udm = np.zeros((HL, NDIAG), dtype=np.float32)
        udm[:, :NDIAG - 1] = np.exp(rel_emb[_BUCKETS, hlo:hhi]).T
        maps.append({
            "xt": np.ascontiguousarray(hidden_states[b].T).astype(bf16),
            "wq": np.ascontiguousarray(Wq[:, hlo * D_KV:hhi * D_KV]).astype(bf16),
            "wk": np.ascontiguousarray(Wk[:, hlo * D_KV:hhi * D_KV]).astype(bf16),
            "wv": np.ascontiguousarray(Wv[:, hlo * D_KV:hhi * D_KV]).astype(bf16),
            "wo": np.ascontiguousarray(Wo[hlo * D_KV:hhi * D_KV, :]).astype(bf16),
            "mask": np.ascontiguousarray(attention_mask[b, 0, 0, :]).astype(np.float32),
            "ud": udm.astype(bf16),
        })
    return maps


def kernel(hidden_states, attention_mask, Wq, Wk, Wv, Wo, rel_emb, _trace=False,
           _trace_kwargs=None):
    hidden_states = np.asarray(hidden_states, dtype=np.float32)
    attention_mask = np.asarray(attention_mask, dtype=np.float32)
    Wq = np.asarray(Wq, dtype=np.float32)
    Wk = np.asarray(Wk, dtype=np.float32)
    Wv = np.asarray(Wv, dtype=np.float32)
    Wo = np.asarray(Wo, dtype=np.float32)
    rel_emb = np.asarray(rel_emb, dtype=np.float32)

    nc = _get_nc()
    maps = _in_maps(hidden_states, attention_mask, Wq, Wk, Wv, Wo, rel_emb)
    kw = dict(_trace_kwargs or {})
    res = run_bass_kernel_spmd(nc, maps, core_ids=list(range(NCORES)),
                               trace=_trace, **kw)
    kernel.last_results = res
    outp = np.empty((B, S, D), dtype=np.float32)
    for b in range(B):
        acc = np.asarray(res.results[4 * b]["out"], dtype=np.float32).copy()
        for g in range(1, 4):
            acc += np.asarray(res.results[4 * b + g]["out"], dtype=np.float32)
        outp[b] = acc
    return outp
